# revision 2
# baseline (speedup 1.0000x reference)
"""Multi-head attention block (QKV proj + softmax attention + out proj) on 8 TRN2 cores.

Sharding: head-parallel. Each core c owns heads (2c, 2c+1) for both batch elements:
  - Wq/Wk/Wv column slice [:, c*128:(c+1)*128], Wo row slice [c*128:(c+1)*128, :]
  - computes Q.T/K.T/V.T for its heads over all 4096 tokens from host-pretransposed X.T
  - attention in transposed layout: S.T[k,q] = K.T^T-free matmuls, softmax along
    partitions handled via an ones-augmented V (denominator rides as output row 64)
  - emits a partial Y.T = Wo_c.T @ O.T ; host sums the 8 partials (row-parallel linear)

All matmuls in bf16 (fp32 PSUM accumulation); softmax exp in fp32 on ScalarE.
"""

import os
import numpy as np
import ml_dtypes

B = 2
S = 2048
TOK = B * S
D = 1024
HD = 64
HC = 128  # head-cols per core: 2 heads x 64
NCORES = 8
KC = D // 128  # contraction chunks for the projections
NKT = S // 128  # k-token tiles per batch
SCALE = 0.125  # 1/sqrt(HD)
QG = 1024  # query-group size (PSUM budget)

_CACHE = {}


def _build_nc():
    import concourse.mybir as mybir
    import concourse.tile as tile
    from concourse import bacc
    from concourse.masks import make_identity

    f32 = mybir.dt.float32
    bf16 = mybir.dt.bfloat16
    Exp = mybir.ActivationFunctionType.Exp

    nc = bacc.Bacc("TRN2", target_bir_lowering=False, debug=False, num_devices=NCORES)
    xt_d = nc.dram_tensor("xt", [D, TOK], bf16, kind="ExternalInput")
    wq_d = nc.dram_tensor("wq", [D, HC], bf16, kind="ExternalInput")
    wk_d = nc.dram_tensor("wk", [D, HC], bf16, kind="ExternalInput")
    wv_d = nc.dram_tensor("wv", [D, HC], bf16, kind="ExternalInput")
    wo_d = nc.dram_tensor("wo", [HC, D], bf16, kind="ExternalInput")
    yt_d = nc.dram_tensor("yt", [D, TOK], f32, kind="ExternalOutput")

    with tile.TileContext(nc) as tc:
        with (
            tc.tile_pool(name="consts", bufs=1) as consts,
            tc.tile_pool(name="persist", bufs=1) as persist,
            tc.tile_pool(name="xqp", bufs=2) as xqp,
            tc.tile_pool(name="ptp", bufs=3) as ptp,
            tc.tile_pool(name="miscp", bufs=2) as miscp,
            tc.tile_pool(name="ysbp", bufs=4) as ysbp,
        ):
            # --- persistent SBUF ---
            w_sb = {}
            for nm, d in (("wq", wq_d), ("wk", wk_d), ("wv", wv_d)):
                w = consts.tile([128, KC, HC], bf16, name=f"{nm}_sb", tag=nm)
                nc.sync.dma_start(w[:], d.rearrange("(o p) m -> p o m", p=128))
                w_sb[nm] = w
            wo_sb = consts.tile([HC, D], bf16, name="wo_sb", tag="wo")
            nc.sync.dma_start(wo_sb[:], wo_d[:])
            ident = consts.tile([128, 128], bf16, name="ident", tag="ident")
            make_identity(nc, ident[:])

            qt = persist.tile([HC, TOK], bf16, name="qt", tag="qt")
            kt = persist.tile([HC, TOK], bf16, name="kt", tag="kt")
            vt = persist.tile([HC, TOK], bf16, name="vt", tag="vt")
            # V' per batch: [tok-part, ktile, 130]; cols 0:64 = V_h0, 64 = ones,
            # 65:129 = V_h1, 129 = ones
            vp = persist.tile([128, B, NKT, 130], bf16, name="vp", tag="vp")
            ot = persist.tile([HC, TOK], bf16, name="ot", tag="ot")
            nc.gpsimd.memset(vp[:], 1.0)

            xt_r = xt_d.rearrange("(o p) n -> p o n", p=128)
            projs = (("q", qt, w_sb["wq"]), ("k", kt, w_sb["wk"]), ("v", vt, w_sb["wv"]))

            # --- phase 1: QKV projections (token quarters) + V transpose ---
            with (
                tc.tile_pool(name="pps", space="PSUM", bufs=2) as pps,
                tc.tile_pool(name="tps", space="PSUM", bufs=2) as tps,
            ):
                for tq in range(4):
                    xq = xqp.tile([128, KC, 1024], bf16, name=f"xq{tq}", tag="xq")
                    nc.sync.dma_start(xq[:], xt_r[:, :, tq * 1024:(tq + 1) * 1024])
                    for pname, dst, w in projs:
                        for nch in range(2):
                            ps = pps.tile([128, 512], f32, name=f"ps_{pname}{tq}{nch}",
                                          tag=f"ps{pname}")
                            for kc in range(KC):
                                nc.tensor.matmul(
                                    ps[:], w[:, kc, :],
                                    xq[:, kc, nch * 512:(nch + 1) * 512],
                                    start=(kc == 0), stop=(kc == KC - 1))
                            c0 = tq * 1024 + nch * 512
                            nc.vector.tensor_copy(out=dst[:, c0:c0 + 512], in_=ps[:])
                for b in range(B):
                    for t in range(NKT):
                        tp = tps.tile([128, 128], bf16, name="tp", tag="tp")
                        nc.tensor.transpose(
                            tp[:], vt[:, b * S + t * 128: b * S + (t + 1) * 128],
                            ident[:])
                        nc.vector.tensor_copy(out=vp[:, b, t, 0:64], in_=tp[:, 0:64])
                        nc.vector.tensor_copy(out=vp[:, b, t, 65:129], in_=tp[:, 64:128])

            # --- phase 2: attention + out-projection per (batch, q-group) ---
            with tc.tile_pool(name="aps", space="PSUM", bufs=1) as aps:
                for b in range(B):
                    for qg in range(S // QG):
                        q0 = b * S + qg * QG
                        # accumulator O'.T: rows 0:64 = O.T_h, row 64 = denom
                        op = aps.tile([65, 2 * QG], f32, name="op", tag="op")
                        for kc in range(NKT):
                            k0 = b * S + kc * 128
                            for h in range(2):
                                sp = aps.tile([128, QG], f32, name="sp", tag="sp",
                                              bufs=2)
                                for nch in range(QG // 512):
                                    nc.tensor.matmul(
                                        sp[:, nch * 512:(nch + 1) * 512],
                                        kt[h * 64:(h + 1) * 64, k0:k0 + 128],
                                        qt[h * 64:(h + 1) * 64,
                                           q0 + nch * 512: q0 + (nch + 1) * 512],
                                        start=True, stop=True)
                                pt = ptp.tile([128, QG], bf16, name="pt", tag="pt")
                                nc.scalar.activation(pt[:], sp[:], Exp, scale=SCALE)
                                for nch in range(QG // 512):
                                    nc.tensor.matmul(
                                        op[:, h * QG + nch * 512: h * QG + (nch + 1) * 512],
                                        vp[:, b, kc, h * 65:(h + 1) * 65],
                                        pt[:, nch * 512:(nch + 1) * 512],
                                        start=(kc == 0), stop=(kc == NKT - 1))
                        # normalize: O.T[:, q] /= denom[q]
                        rr = miscp.tile([1, 2 * QG], f32, name="rr", tag="rr")
                        nc.vector.reciprocal(rr[:], op[64:65, :])
                        rb = miscp.tile([64, 2 * QG], f32, name="rb", tag="rb")
                        nc.gpsimd.partition_broadcast(rb[:], rr[:])
                        for h in range(2):
                            nc.vector.tensor_mul(
                                out=ot[h * 64:(h + 1) * 64, q0:q0 + QG],
                                in0=op[0:64, h * QG:(h + 1) * QG],
                                in1=rb[:, h * QG:(h + 1) * QG])
                        # out-projection for this token group
                        for od in range(8):
                            yp = aps.tile([128, QG], f32, name="yp", tag="sp", bufs=2)
                            for tch in range(QG // 512):
                                nc.tensor.matmul(
                                    yp[:, tch * 512:(tch + 1) * 512],
                                    wo_sb[:, od * 128:(od + 1) * 128],
                                    ot[:, q0 + tch * 512: q0 + (tch + 1) * 512],
                                    start=True, stop=True)
                            ysb = ysbp.tile([128, QG], f32, name="ysb", tag="ysb")
                            nc.vector.tensor_copy(out=ysb[:], in_=yp[:])
                            nc.sync.dma_start(
                                yt_d[od * 128:(od + 1) * 128, q0:q0 + QG], ysb[:])
    nc.compile()
    return nc


def get_nc():
    if "nc" not in _CACHE:
        _CACHE["nc"] = _build_nc()
    return _CACHE["nc"]


def make_in_maps(hidden_states, Wq, Wk, Wv, Wo):
    bf = ml_dtypes.bfloat16
    X = np.ascontiguousarray(np.asarray(hidden_states, np.float32).reshape(TOK, D))
    xt = np.ascontiguousarray(X.T).astype(bf)
    Wq = np.asarray(Wq, np.float32)
    Wk = np.asarray(Wk, np.float32)
    Wv = np.asarray(Wv, np.float32)
    Wo = np.asarray(Wo, np.float32)
    in_maps = []
    for c in range(NCORES):
        sl = slice(c * HC, (c + 1) * HC)
        in_maps.append({
            "xt": xt,
            "wq": np.ascontiguousarray(Wq[:, sl]).astype(bf),
            "wk": np.ascontiguousarray(Wk[:, sl]).astype(bf),
            "wv": np.ascontiguousarray(Wv[:, sl]).astype(bf),
            "wo": np.ascontiguousarray(Wo[sl, :]).astype(bf),
        })
    return in_maps


def kernel(hidden_states, Wq, Wk, Wv, Wo, bo):
    from concourse.bass_utils import run_bass_kernel_spmd

    nc = get_nc()
    in_maps = make_in_maps(hidden_states, Wq, Wk, Wv, Wo)
    res = run_bass_kernel_spmd(nc, in_maps, list(range(NCORES)))
    _CACHE["last_result"] = res
    yt = np.zeros((D, TOK), np.float32)
    for c in range(NCORES):
        yt += res.results[c]["yt"]
    out = yt.T.reshape(B, S, D) + np.asarray(bo, np.float32)[None, None, :]
    return out.astype(np.float32)


# revision 31
# speedup vs baseline: 1.4425x; 1.4425x over previous
"""Multi-head attention block (QKV proj + softmax attention + out proj) on 8 TRN2 cores.

Sharding: head-parallel. Each core c owns heads (2c, 2c+1) for both batch elements:
  - Wq/Wk/Wv column slice [:, c*128:(c+1)*128], Wo row slice [c*128:(c+1)*128, :]
  - computes Q.T/K.T/V.T for its heads over all 4096 tokens from host-pretransposed X.T
  - attention in transposed layout: S.T[k,q] tiles; softmax denominator rides as an
    extra all-ones column of V (output row 64 of the PV matmul), so no partition
    reduction is ever needed; normalization deferred to after PV.
  - emits a partial Y = O @ Wo_c ; host sums the 8 partials (row-parallel linear)

All matmuls in bf16 (fp32 PSUM accumulation); softmax exp in fp32 on ScalarE with
the 1/sqrt(d) scale folded into the activation's affine pre-scale.

Emission is interleaved (generators) so the batch-1 projections execute in the
PE gaps of batch-0's ACT-bound attention, and out-projections overlap the next
attention phases. PSUM budget (8 banks): sp [128,1024]x2 (4) + op [65,1024]x1 (2)
+ yp/proj/vtrans [128,512]x2 (2).
"""

import numpy as np
import ml_dtypes

B = 2
S = 2048
TOK = B * S
D = 1024
HD = 64
HC = 128  # head-cols per core: 2 heads x 64
NCORES = 8
KC = D // 128  # contraction chunks for the projections
NKT = S // 128  # k-token tiles per batch
SCALE = 0.125  # 1/sqrt(HD)
QG = 512  # query-group size per attention phase
NQG = S // QG  # phases per batch

_CACHE = {}


def _build_nc():
    import concourse.mybir as mybir
    import concourse.tile as tile
    from concourse import bacc
    from concourse.masks import make_identity

    f32 = mybir.dt.float32
    bf16 = mybir.dt.bfloat16
    Exp = mybir.ActivationFunctionType.Exp

    nc = bacc.Bacc("TRN2", target_bir_lowering=False, debug=False, num_devices=NCORES)
    xt_d = nc.dram_tensor("xt", [D, TOK], bf16, kind="ExternalInput")
    wq_d = nc.dram_tensor("wq", [D, HC], bf16, kind="ExternalInput")
    wk_d = nc.dram_tensor("wk", [D, HC], bf16, kind="ExternalInput")
    wv_d = nc.dram_tensor("wv", [D, HC], bf16, kind="ExternalInput")
    wo_d = nc.dram_tensor("wo", [HC, D], bf16, kind="ExternalInput")
    y_d = nc.dram_tensor("y", [TOK, D], f32, kind="ExternalOutput")

    with tile.TileContext(nc) as tc:
        with (
            tc.tile_pool(name="consts", bufs=1) as consts,
            tc.tile_pool(name="persist", bufs=1) as persist,
            tc.tile_pool(name="xqp", bufs=2) as xqp,
            tc.tile_pool(name="ptp", bufs=3) as ptp,
            tc.tile_pool(name="miscp", bufs=2) as miscp,
            tc.tile_pool(name="ysbp", bufs=6) as ysbp,
            tc.tile_pool(name="aps", space="PSUM", bufs=1) as aps,
        ):
            # --- persistent SBUF ---
            w_sb = {}
            for nm, d, eng in (("wk", wk_d, nc.sync), ("wq", wq_d, nc.scalar),
                               ("wv", wv_d, nc.scalar)):
                w = consts.tile([128, KC, HC], bf16, name=f"{nm}_sb", tag=nm)
                eng.dma_start(w[:], d.rearrange("(o p) m -> p o m", p=128))
                w_sb[nm] = w
            wo_sb = consts.tile([HC, D], bf16, name="wo_sb", tag="wo")
            nc.scalar.dma_start(wo_sb[:], wo_d[:])
            ident = consts.tile([128, 128], bf16, name="ident", tag="ident")
            make_identity(nc, ident[:])

            qt = persist.tile([HC, TOK], bf16, name="qt", tag="qt")
            kt = persist.tile([HC, TOK], bf16, name="kt", tag="kt")
            vt = persist.tile([HC, TOK], bf16, name="vt", tag="vt")
            # V' per batch: [tok-part, ktile, 130]; cols 0:64 = V_h0, col 64 =
            # ones, 65:129 = V_h1, col 129 = ones. h lhsT = cols h*65:h*65+65;
            # denominator lands in out row 64 for both heads.
            vp = persist.tile([128, B, NKT, 130], bf16, name="vp", tag="vp")
            ot = persist.tile([HC, TOK], bf16, name="ot", tag="ot")
            nc.gpsimd.memset(vp[:, :, :, 64:65], 1.0)
            nc.gpsimd.memset(vp[:, :, :, 129:130], 1.0)

            xt_r = xt_d.rearrange("(o p) n -> p o n", p=128)
            xq_tiles = {}

            dma_engs = [nc.sync, nc.scalar, nc.gpsimd]

            def load_xq(tq):
                xq = xqp.tile([128, KC, 1024], bf16, name=f"xq{tq}", tag="xq", bufs=2)
                if tq < 2:
                    # region-0 load: split per chunk across idle queues
                    for kc in range(KC):
                        eng = nc.sync if kc % 2 == 0 else nc.scalar
                        eng.dma_start(xq[:, kc:kc + 1, :],
                                      xt_r[:, kc:kc + 1, tq * 1024:(tq + 1) * 1024])
                else:
                    nc.sync.dma_start(xq[:], xt_r[:, :, tq * 1024:(tq + 1) * 1024])
                xq_tiles[tq] = xq

            # warm the ACT exp table off the critical path
            warm = miscp.tile([1, 64], f32, name="warm", tag="warm", bufs=1)
            nc.gpsimd.memset(warm[:], 0.0)
            nc.scalar.activation(warm[:], warm[:], Exp)

            projs = {"q": (qt, w_sb["wq"]), "k": (kt, w_sb["wk"]), "v": (vt, w_sb["wv"])}

            def proj_group(tq, pname, nch):
                """One [128,512] projection output; yields after each matmul."""
                dst, w = projs[pname]
                xq = xq_tiles[tq]
                ps = aps.tile([128, 512], f32, name=f"ps_{pname}{tq}{nch}", tag="yp",
                              bufs=2)
                for kc in range(KC):
                    nc.tensor.matmul(ps[:], w[:, kc, :],
                                     xq[:, kc, nch * 512:(nch + 1) * 512],
                                     start=(kc == 0), stop=(kc == KC - 1))
                    yield
                c0 = tq * 1024 + nch * 512
                nc.vector.tensor_copy(out=dst[:, c0:c0 + 512], in_=ps[:])
                yield

            def vtrans_unit(b, t):
                tp = aps.tile([128, 128], bf16, name="tp", tag="yp", bufs=2)
                nc.tensor.transpose(
                    tp[:], vt[:, b * S + t * 128: b * S + (t + 1) * 128], ident[:])
                nc.vector.tensor_copy(out=vp[:, b, t, 0:64], in_=tp[:, 0:64])
                nc.vector.tensor_copy(out=vp[:, b, t, 65:129], in_=tp[:, 64:128])
                yield

            def outproj_unit(b, qg, tt):
                """Y[tok-tile, :] for one 128-token tile (lhsT=O.T reused across od)."""
                q0 = b * S + qg * QG
                t0 = q0 + tt * 128
                for odc in range(2):
                    yp = aps.tile([128, 512], f32, name="yp", tag="yp", bufs=2)
                    nc.tensor.matmul(yp[:], ot[:, t0:t0 + 128],
                                     wo_sb[:, odc * 512:(odc + 1) * 512],
                                     start=True, stop=True)
                    ysb = ysbp.tile([128, 512], f32, name="ysb", tag="ysb")
                    nc.vector.tensor_copy(out=ysb[:], in_=yp[:])
                    nc.sync.dma_start(
                        y_d[t0:t0 + 128, odc * 512:(odc + 1) * 512], ysb[:])
                yield

            outproj_pending = []

            def attention_phase(b, qg, fill, pre=None):
                """One (batch, 512-query-group) phase; pulls from `fill` each kc.

                Software-pipelined: scores(kc+1) is emitted before attnV(kc) so
                the PE stays one step ahead of ACT and exp never waits. `pre(kc)`
                (if given) emits units that must precede attnV(kc) on the PE.
                """
                q0 = b * S + qg * QG

                def scores(kc):
                    k0 = b * S + kc * 128
                    sp = aps.tile([128, 2 * QG], f32, name="sp", tag="sp", bufs=2)
                    for h in range(2):
                        nc.tensor.matmul(
                            sp[:, h * QG:(h + 1) * QG],
                            kt[h * 64:(h + 1) * 64, k0:k0 + 128],
                            qt[h * 64:(h + 1) * 64, q0:q0 + QG],
                            start=True, stop=True)
                    return sp

                op = aps.tile([65, 2 * QG], f32, name="op", tag="op", bufs=1)
                sp_cur = scores(0)
                for kc in range(NKT):
                    sp_next = scores(kc + 1) if kc + 1 < NKT else None
                    pt = ptp.tile([128, 2 * QG], bf16, name="pt", tag="pt", bufs=3)
                    nc.scalar.activation(pt[:], sp_cur[:], Exp, scale=SCALE)
                    if pre is not None:
                        pre(kc)
                    for h in range(2):
                        nc.tensor.matmul(
                            op[:, h * QG:(h + 1) * QG],
                            vp[:, b, kc, h * 65:h * 65 + 65],
                            pt[:, h * QG:(h + 1) * QG],
                            start=(kc == 0), stop=(kc == NKT - 1))
                    sp_cur = sp_next
                    fill(kc)
                # stage op to SBUF (frees the PSUM slot for the next phase),
                # then normalize: O.T[:, q] /= denom[q] (denom = row 64, both heads)
                osb = miscp.tile([65, 2 * QG], f32, name="osb", tag="osb", bufs=2)
                nc.vector.tensor_copy(out=osb[:], in_=op[:])
                rr = miscp.tile([1, 2 * QG], f32, name="rr", tag="rr", bufs=2)
                nc.vector.reciprocal(rr[:], osb[64:65, :])
                rb = miscp.tile([64, 2 * QG], f32, name="rb", tag="rb", bufs=2)
                nc.gpsimd.partition_broadcast(rb[:], rr[:])
                for h in range(2):
                    nc.vector.tensor_mul(
                        out=ot[h * 64:(h + 1) * 64, q0:q0 + QG],
                        in0=osb[0:64, h * QG:(h + 1) * QG],
                        in1=rb[:, h * QG:(h + 1) * QG])
                outproj_pending.extend(
                    outproj_unit(b, qg, tt) for tt in range(QG // 128))

            def make_fill(stream, steps_per_call):
                state = {"it": iter(stream), "gen": None}

                def step():
                    """Advance the stream by one emitted chunk; False when done."""
                    while True:
                        if state["gen"] is None:
                            state["gen"] = next(state["it"], None)
                            if state["gen"] is None:
                                return False
                        if next(state["gen"], StopIteration) is StopIteration:
                            state["gen"] = None
                            continue
                        return True

                def fill(_kc):
                    for _ in range(steps_per_call):
                        if not step():
                            return

                def drain():
                    while step():
                        pass
                return fill, drain

            # ---- final structure: fragmented cross-phase/cross-region fill ----
            load_xq(0)
            load_xq(1)
            for p, nch in (("k", 0), ("k", 1), ("v", 0), ("v", 1), ("q", 0)):
                for _ in proj_group(0, p, nch):
                    pass
            for t in range(NKT // 2):
                for _ in vtrans_unit(0, t):
                    pass
            for p in ("k", "v"):
                for nch in range(2):
                    for _ in proj_group(1, p, nch):
                        pass
            for t in range(NKT // 2, NKT):
                for _ in vtrans_unit(0, t):
                    pass

            load_xq(2)
            load_xq(3)
            fill1, drain1 = make_fill(
                [proj_group(0, "q", 1), proj_group(1, "q", 0), proj_group(1, "q", 1)]
                + [proj_group(2, p, nch) for p in ("k", "v") for nch in range(2)]
                + [proj_group(2, "q", 0)]
                + [proj_group(3, p, nch) for p in ("k", "v") for nch in range(2)]
                + [vtrans_unit(1, t) for t in range(NKT)]
                + [proj_group(2, "q", 1), proj_group(3, "q", 0),
                   proj_group(3, "q", 1)],
                steps_per_call=3)
            for qg in range(NQG):
                attention_phase(0, qg, fill1)
            drain1()

            def fill2(kc):
                if kc % 2 == 0 and outproj_pending:
                    for _ in outproj_pending.pop(0):
                        pass
            for qg in range(NQG):
                attention_phase(1, qg, fill2)
            while outproj_pending:
                for _ in outproj_pending.pop(0):
                    pass
    nc.compile()
    return nc


def get_nc():
    if "nc" not in _CACHE:
        _CACHE["nc"] = _build_nc()
    return _CACHE["nc"]


def make_in_maps(hidden_states, Wq, Wk, Wv, Wo):
    bf = ml_dtypes.bfloat16
    X = np.ascontiguousarray(np.asarray(hidden_states, np.float32).reshape(TOK, D))
    xt = np.ascontiguousarray(X.T).astype(bf)
    Wq = np.asarray(Wq, np.float32)
    Wk = np.asarray(Wk, np.float32)
    Wv = np.asarray(Wv, np.float32)
    Wo = np.asarray(Wo, np.float32)
    in_maps = []
    for c in range(NCORES):
        sl = slice(c * HC, (c + 1) * HC)
        in_maps.append({
            "xt": xt,
            "wq": np.ascontiguousarray(Wq[:, sl]).astype(bf),
            "wk": np.ascontiguousarray(Wk[:, sl]).astype(bf),
            "wv": np.ascontiguousarray(Wv[:, sl]).astype(bf),
            "wo": np.ascontiguousarray(Wo[sl, :]).astype(bf),
        })
    return in_maps


def kernel(hidden_states, Wq, Wk, Wv, Wo, bo):
    from concourse.bass_utils import run_bass_kernel_spmd

    nc = get_nc()
    in_maps = make_in_maps(hidden_states, Wq, Wk, Wv, Wo)
    res = run_bass_kernel_spmd(nc, in_maps, list(range(NCORES)))
    _CACHE["last_result"] = res
    y = np.zeros((TOK, D), np.float32)
    for c in range(NCORES):
        y += res.results[c]["y"]
    out = y.reshape(B, S, D) + np.asarray(bo, np.float32)[None, None, :]
    return out.astype(np.float32)


# revision 36
# speedup vs baseline: 1.5135x; 1.0492x over previous
"""Multi-head attention block (QKV proj + softmax attention + out proj) on 8 TRN2 cores.

Sharding: head-parallel. Each core c owns heads (2c, 2c+1) for both batch elements:
  - Wq/Wk/Wv column slice [:, c*128:(c+1)*128], Wo row slice [c*128:(c+1)*128, :]
  - computes Q.T/K.T/V.T for its heads over all 4096 tokens from host-pretransposed X.T
  - attention in transposed layout: S.T[k,q] tiles; softmax denominator rides as an
    extra all-ones column of V (output row 64 of the PV matmul), so no partition
    reduction is ever needed; normalization deferred to after PV.
  - emits a partial Y = O @ Wo_c ; host sums the 8 partials (row-parallel linear)

All matmuls in bf16 (fp32 PSUM accumulation); softmax exp in fp32 on ScalarE with
the 1/sqrt(d) scale folded into the activation's affine pre-scale.

Emission is interleaved (generators) so the batch-1 projections execute in the
PE gaps of batch-0's ACT-bound attention, and out-projections overlap the
batch-1 attention phases. PSUM budget (8 banks): sp [128,1024]x2 (4) +
op [65,1024]x1 (2) + yp/proj/vtrans [128,512]x2 (2).

Hardware constraint discovered by A/B testing (invisible to CoreSim, flaky on
HW): an interleaved unit must be fully emitted BEFORE the attention phase that
consumes it starts; same-phase delivery (e.g., vtrans for tile kc emitted a
couple of kc-iterations ahead of its attnV) produces stale reads on silicon.
All fill streams here only carry units consumed by later-emitted phases.
"""

import numpy as np
import ml_dtypes

B = 2
S = 2048
TOK = B * S
D = 1024
HD = 64
HC = 128  # head-cols per core: 2 heads x 64
NCORES = 8
KC = D // 128  # contraction chunks for the projections
NKT = S // 128  # k-token tiles per batch
SCALE = 0.125  # 1/sqrt(HD)
QG = 512  # query-group size per attention phase
NQG = S // QG  # phases per batch

_CACHE = {}


def _build_nc():
    import concourse.mybir as mybir
    import concourse.tile as tile
    from concourse import bacc
    from concourse.masks import make_identity

    f32 = mybir.dt.float32
    bf16 = mybir.dt.bfloat16
    Exp = mybir.ActivationFunctionType.Exp

    nc = bacc.Bacc("TRN2", target_bir_lowering=False, debug=False, num_devices=NCORES)
    xt_d = nc.dram_tensor("xt", [D, TOK], bf16, kind="ExternalInput")
    wq_d = nc.dram_tensor("wq", [D, HC], bf16, kind="ExternalInput")
    wk_d = nc.dram_tensor("wk", [D, HC], bf16, kind="ExternalInput")
    wv_d = nc.dram_tensor("wv", [D, HC], bf16, kind="ExternalInput")
    wo_d = nc.dram_tensor("wo", [HC, D], bf16, kind="ExternalInput")
    y_d = nc.dram_tensor("y", [TOK, D], f32, kind="ExternalOutput")

    with tile.TileContext(nc) as tc:
        with (
            tc.tile_pool(name="consts", bufs=1) as consts,
            tc.tile_pool(name="persist", bufs=1) as persist,
            tc.tile_pool(name="xqp", bufs=2) as xqp,
            tc.tile_pool(name="ptp", bufs=3) as ptp,
            tc.tile_pool(name="miscp", bufs=2) as miscp,
            tc.tile_pool(name="ysbp", bufs=6) as ysbp,
            tc.tile_pool(name="aps", space="PSUM", bufs=1) as aps,
        ):
            # --- persistent SBUF ---
            w_sb = {}

            def load_weight(nm, d, eng):
                w = consts.tile([128, KC, HC], bf16, name=f"{nm}_sb", tag=nm)
                eng.dma_start(w[:], d.rearrange("(o p) m -> p o m", p=128))
                w_sb[nm] = w
            load_weight("wk", wk_d, nc.sync)
            wo_sb = consts.tile([HC, D], bf16, name="wo_sb", tag="wo")
            ident = consts.tile([128, 128], bf16, name="ident", tag="ident")
            make_identity(nc, ident[:])

            qt = persist.tile([HC, TOK], bf16, name="qt", tag="qt")
            kt = persist.tile([HC, TOK], bf16, name="kt", tag="kt")
            vt = persist.tile([HC, TOK], bf16, name="vt", tag="vt")
            # V' per batch: [tok-part, ktile, 130]; cols 0:64 = V_h0, col 64 =
            # ones, 65:129 = V_h1, col 129 = ones. h lhsT = cols h*65:h*65+65;
            # denominator lands in out row 64 for both heads.
            vp = persist.tile([128, B, NKT, 130], bf16, name="vp", tag="vp")
            ot = persist.tile([HC, TOK], bf16, name="ot", tag="ot")
            nc.gpsimd.memset(vp[:, :, :, 64:65], 1.0)
            nc.gpsimd.memset(vp[:, :, :, 129:130], 1.0)

            xt_r = xt_d.rearrange("(o p) n -> p o n", p=128)
            xq_tiles = {}

            def load_xq(tq):
                xq = xqp.tile([128, KC, 1024], bf16, name=f"xq{tq}", tag="xq", bufs=2)
                if tq < 2:
                    # region-0 load: split per chunk across idle queues
                    for kc in range(KC):
                        eng = nc.sync if kc % 2 == 0 else nc.scalar
                        eng.dma_start(xq[:, kc:kc + 1, :],
                                      xt_r[:, kc:kc + 1, tq * 1024:(tq + 1) * 1024])
                else:
                    nc.sync.dma_start(xq[:], xt_r[:, :, tq * 1024:(tq + 1) * 1024])
                xq_tiles[tq] = xq

            # warm the ACT exp table off the critical path
            warm = miscp.tile([1, 64], f32, name="warm", tag="warm", bufs=1)
            nc.gpsimd.memset(warm[:], 0.0)
            nc.scalar.activation(warm[:], warm[:], Exp)

            proj_dst = {"q": qt, "k": kt, "v": vt}

            def proj_group(tq, pname, nch):
                """One [128,512] projection output; yields after each matmul."""
                dst, w = proj_dst[pname], w_sb["w" + pname]
                xq = xq_tiles[tq]
                ps = aps.tile([128, 512], f32, name=f"ps_{pname}{tq}{nch}", tag="yp",
                              bufs=2)
                for kc in range(KC):
                    nc.tensor.matmul(ps[:], w[:, kc, :],
                                     xq[:, kc, nch * 512:(nch + 1) * 512],
                                     start=(kc == 0), stop=(kc == KC - 1))
                    yield
                c0 = tq * 1024 + nch * 512
                nc.vector.tensor_copy(out=dst[:, c0:c0 + 512], in_=ps[:])
                yield

            def vtrans_unit(b, t):
                tp = aps.tile([128, 128], bf16, name="tp", tag="yp", bufs=2)
                nc.tensor.transpose(
                    tp[:], vt[:, b * S + t * 128: b * S + (t + 1) * 128], ident[:])
                nc.vector.tensor_copy(out=vp[:, b, t, 0:64], in_=tp[:, 0:64])
                nc.vector.tensor_copy(out=vp[:, b, t, 65:129], in_=tp[:, 64:128])
                yield

            def outproj_unit(b, qg, tt):
                """Y[tok-tile, :] for one 128-token tile (lhsT=O.T reused across od)."""
                q0 = b * S + qg * QG
                t0 = q0 + tt * 128
                for odc in range(2):
                    yp = aps.tile([128, 512], f32, name="yp", tag="yp", bufs=2)
                    nc.tensor.matmul(yp[:], ot[:, t0:t0 + 128],
                                     wo_sb[:, odc * 512:(odc + 1) * 512],
                                     start=True, stop=True)
                    ysb = ysbp.tile([128, 512], f32, name="ysb", tag="ysb")
                    nc.vector.tensor_copy(out=ysb[:], in_=yp[:])
                    nc.sync.dma_start(
                        y_d[t0:t0 + 128, odc * 512:(odc + 1) * 512], ysb[:])
                yield

            outproj_pending = []

            def attention_phase(b, qg, fill, pre=None):
                """One (batch, 512-query-group) phase; pulls from `fill` each kc.

                Software-pipelined: scores(kc+1) is emitted before attnV(kc) so
                the PE stays one step ahead of ACT and exp never waits. `pre(kc)`
                (if given) emits units that must precede attnV(kc) on the PE.
                """
                q0 = b * S + qg * QG

                def scores(kc):
                    k0 = b * S + kc * 128
                    sp = aps.tile([128, 2 * QG], f32, name="sp", tag="sp", bufs=2)
                    for h in range(2):
                        nc.tensor.matmul(
                            sp[:, h * QG:(h + 1) * QG],
                            kt[h * 64:(h + 1) * 64, k0:k0 + 128],
                            qt[h * 64:(h + 1) * 64, q0:q0 + QG],
                            start=True, stop=True)
                    return sp

                op = aps.tile([65, 2 * QG], f32, name="op", tag="op", bufs=1)
                sp_cur = scores(0)
                for kc in range(NKT):
                    sp_next = scores(kc + 1) if kc + 1 < NKT else None
                    pt = ptp.tile([128, 2 * QG], bf16, name="pt", tag="pt", bufs=3)
                    nc.scalar.activation(pt[:], sp_cur[:], Exp, scale=SCALE)
                    if pre is not None:
                        pre(kc)
                    for h in range(2):
                        nc.tensor.matmul(
                            op[:, h * QG:(h + 1) * QG],
                            vp[:, b, kc, h * 65:h * 65 + 65],
                            pt[:, h * QG:(h + 1) * QG],
                            start=(kc == 0), stop=(kc == NKT - 1))
                    sp_cur = sp_next
                    fill(kc)
                # stage op to SBUF (frees the PSUM slot for the next phase),
                # then normalize: O.T[:, q] /= denom[q] (denom = row 64, both heads)
                osb = miscp.tile([65, 2 * QG], f32, name="osb", tag="osb", bufs=2)
                nc.vector.tensor_copy(out=osb[:], in_=op[:])
                rr = miscp.tile([1, 2 * QG], f32, name="rr", tag="rr", bufs=2)
                nc.vector.reciprocal(rr[:], osb[64:65, :])
                rb = miscp.tile([64, 2 * QG], f32, name="rb", tag="rb", bufs=2)
                nc.gpsimd.partition_broadcast(rb[:], rr[:])
                for h in range(2):
                    nc.vector.tensor_mul(
                        out=ot[h * 64:(h + 1) * 64, q0:q0 + QG],
                        in0=osb[0:64, h * QG:(h + 1) * QG],
                        in1=rb[:, h * QG:(h + 1) * QG])
                outproj_pending.extend(
                    outproj_unit(b, qg, tt) for tt in range(QG // 128))

            def make_fill(stream, steps_per_call):
                state = {"it": iter(stream), "gen": None}

                def step():
                    """Advance the stream by one emitted chunk; False when done."""
                    while True:
                        if state["gen"] is None:
                            state["gen"] = next(state["it"], None)
                            if state["gen"] is None:
                                return False
                        if next(state["gen"], StopIteration) is StopIteration:
                            state["gen"] = None
                            continue
                        return True

                def fill(_kc):
                    for _ in range(steps_per_call):
                        if not step():
                            return

                def drain():
                    while step():
                        pass
                return fill, drain

            # ---- final structure: fragmented cross-phase/cross-region fill ----
            load_xq(0)
            load_weight("wv", wv_d, nc.scalar)
            load_weight("wq", wq_d, nc.scalar)
            load_xq(1)
            nc.scalar.dma_start(wo_sb[:], wo_d[:])
            for p, nch in (("k", 0), ("k", 1), ("v", 0), ("v", 1), ("q", 0)):
                for _ in proj_group(0, p, nch):
                    pass
            for t in range(NKT // 2):
                for _ in vtrans_unit(0, t):
                    pass
            for p in ("k", "v"):
                for nch in range(2):
                    for _ in proj_group(1, p, nch):
                        pass
            for t in range(NKT // 2, NKT):
                for _ in vtrans_unit(0, t):
                    pass

            load_xq(2)
            load_xq(3)
            fill1, drain1 = make_fill(
                [proj_group(0, "q", 1), proj_group(1, "q", 0), proj_group(1, "q", 1)]
                + [proj_group(2, p, nch) for p in ("k", "v") for nch in range(2)]
                + [proj_group(2, "q", 0)]
                + [proj_group(3, p, nch) for p in ("k", "v") for nch in range(2)]
                + [vtrans_unit(1, t) for t in range(NKT)]
                + [proj_group(2, "q", 1), proj_group(3, "q", 0),
                   proj_group(3, "q", 1)],
                steps_per_call=3)
            for qg in range(NQG):
                attention_phase(0, qg, fill1)
            drain1()

            def fill2(kc):
                if kc % 2 == 0 and outproj_pending:
                    for _ in outproj_pending.pop(0):
                        pass
            for qg in range(NQG):
                attention_phase(1, qg, fill2)
            while outproj_pending:
                for _ in outproj_pending.pop(0):
                    pass
    nc.compile()
    return nc


def get_nc():
    if "nc" not in _CACHE:
        _CACHE["nc"] = _build_nc()
    return _CACHE["nc"]


def make_in_maps(hidden_states, Wq, Wk, Wv, Wo):
    bf = ml_dtypes.bfloat16
    X = np.ascontiguousarray(np.asarray(hidden_states, np.float32).reshape(TOK, D))
    xt = np.ascontiguousarray(X.T).astype(bf)
    Wq = np.asarray(Wq, np.float32)
    Wk = np.asarray(Wk, np.float32)
    Wv = np.asarray(Wv, np.float32)
    Wo = np.asarray(Wo, np.float32)
    in_maps = []
    for c in range(NCORES):
        sl = slice(c * HC, (c + 1) * HC)
        in_maps.append({
            "xt": xt,
            "wq": np.ascontiguousarray(Wq[:, sl]).astype(bf),
            "wk": np.ascontiguousarray(Wk[:, sl]).astype(bf),
            "wv": np.ascontiguousarray(Wv[:, sl]).astype(bf),
            "wo": np.ascontiguousarray(Wo[sl, :]).astype(bf),
        })
    return in_maps


def kernel(hidden_states, Wq, Wk, Wv, Wo, bo):
    from concourse.bass_utils import run_bass_kernel_spmd

    nc = get_nc()
    in_maps = make_in_maps(hidden_states, Wq, Wk, Wv, Wo)
    res = run_bass_kernel_spmd(nc, in_maps, list(range(NCORES)))
    _CACHE["last_result"] = res
    y = np.zeros((TOK, D), np.float32)
    for c in range(NCORES):
        y += res.results[c]["y"]
    out = y.reshape(B, S, D) + np.asarray(bo, np.float32)[None, None, :]
    return out.astype(np.float32)


# revision 39
# speedup vs baseline: 1.5371x; 1.0156x over previous
"""Multi-head attention block (QKV proj + softmax attention + out proj) on 8 TRN2 cores.

Sharding: head-parallel. Each core c owns heads (2c, 2c+1) for both batch elements:
  - Wq/Wk/Wv column slice [:, c*128:(c+1)*128], Wo row slice [c*128:(c+1)*128, :]
  - computes Q.T/K.T/V.T for its heads over all 4096 tokens from host-pretransposed X.T
  - attention in transposed layout: S.T[k,q] tiles; softmax denominator rides as an
    extra all-ones column of V (output row 64 of the PV matmul), so no partition
    reduction is ever needed; normalization deferred to after PV.
  - emits a partial Y = O @ Wo_c ; host sums the 8 partials (row-parallel linear)

All matmuls in bf16 (fp32 PSUM accumulation); softmax exp in fp32 on ScalarE with
the 1/sqrt(d) scale folded into the activation's affine pre-scale.

Emission is interleaved (generators) so the batch-1 projections execute in the
PE gaps of batch-0's ACT-bound attention, and out-projections overlap the
batch-1 attention phases. PSUM budget (8 banks): sp [128,1024]x2 (4) +
op [65,1024]x1 (2) + yp/proj/vtrans [128,512]x2 (2).

Hardware constraint discovered by A/B testing (invisible to CoreSim, flaky on
HW): an interleaved unit must be fully emitted BEFORE the attention phase that
consumes it starts; same-phase delivery (e.g., vtrans for tile kc emitted a
couple of kc-iterations ahead of its attnV) produces stale reads on silicon.
All fill streams here only carry units consumed by later-emitted phases.
"""

import numpy as np
import ml_dtypes

B = 2
S = 2048
TOK = B * S
D = 1024
HD = 64
HC = 128  # head-cols per core: 2 heads x 64
NCORES = 8
KC = D // 128  # contraction chunks for the projections
NKT = S // 128  # k-token tiles per batch
SCALE = 0.125  # 1/sqrt(HD)
QG = 512  # query-group size per attention phase
NQG = S // QG  # phases per batch

_CACHE = {}


def _build_nc():
    import concourse.mybir as mybir
    import concourse.tile as tile
    from concourse import bacc
    from concourse.masks import make_identity

    f32 = mybir.dt.float32
    bf16 = mybir.dt.bfloat16
    Exp = mybir.ActivationFunctionType.Exp

    nc = bacc.Bacc("TRN2", target_bir_lowering=False, debug=False, num_devices=NCORES)
    xt_d = nc.dram_tensor("xt", [D, TOK], bf16, kind="ExternalInput")
    wq_d = nc.dram_tensor("wq", [D, HC], bf16, kind="ExternalInput")
    wk_d = nc.dram_tensor("wk", [D, HC], bf16, kind="ExternalInput")
    wv_d = nc.dram_tensor("wv", [D, HC], bf16, kind="ExternalInput")
    wo_d = nc.dram_tensor("wo", [HC, D], bf16, kind="ExternalInput")
    y_d = nc.dram_tensor("y", [TOK, D], f32, kind="ExternalOutput")

    with tile.TileContext(nc) as tc:
        with (
            tc.tile_pool(name="consts", bufs=1) as consts,
            tc.tile_pool(name="persist", bufs=1) as persist,
            tc.tile_pool(name="xqp", bufs=2) as xqp,
            tc.tile_pool(name="ptp", bufs=3) as ptp,
            tc.tile_pool(name="miscp", bufs=2) as miscp,
            tc.tile_pool(name="ysbp", bufs=6) as ysbp,
            tc.tile_pool(name="aps", space="PSUM", bufs=1) as aps,
        ):
            # --- persistent SBUF ---
            w_sb = {}

            def load_weight(nm, d, eng):
                w = consts.tile([128, KC, HC], bf16, name=f"{nm}_sb", tag=nm)
                eng.dma_start(w[:], d.rearrange("(o p) m -> p o m", p=128))
                w_sb[nm] = w
            load_weight("wk", wk_d, nc.sync)
            wo_sb = consts.tile([HC, D], bf16, name="wo_sb", tag="wo")
            ident = consts.tile([128, 128], bf16, name="ident", tag="ident")
            make_identity(nc, ident[:])

            qt = persist.tile([HC, TOK], bf16, name="qt", tag="qt")
            kt = persist.tile([HC, TOK], bf16, name="kt", tag="kt")
            vt = persist.tile([HC, TOK], bf16, name="vt", tag="vt")
            # V' per batch: [tok-part, ktile, 130]; cols 0:64 = V_h0, col 64 =
            # ones, 65:129 = V_h1, col 129 = ones. h lhsT = cols h*65:h*65+65;
            # denominator lands in out row 64 for both heads.
            vp = persist.tile([128, B, NKT, 130], bf16, name="vp", tag="vp")
            ot = persist.tile([HC, TOK], bf16, name="ot", tag="ot")
            nc.gpsimd.memset(vp[:, :, :, 64:65], 1.0)
            nc.gpsimd.memset(vp[:, :, :, 129:130], 1.0)

            xt_r = xt_d.rearrange("(o p) n -> p o n", p=128)
            xq_tiles = {}

            def load_xq(tq):
                xq = xqp.tile([128, KC, 1024], bf16, name=f"xq{tq}", tag="xq", bufs=2)
                if tq < 2:
                    # region-0 load: split per chunk across idle queues
                    for kc in range(KC):
                        eng = nc.sync if kc % 2 == 0 else nc.scalar
                        eng.dma_start(xq[:, kc:kc + 1, :],
                                      xt_r[:, kc:kc + 1, tq * 1024:(tq + 1) * 1024])
                else:
                    nc.sync.dma_start(xq[:], xt_r[:, :, tq * 1024:(tq + 1) * 1024])
                xq_tiles[tq] = xq

            # warm the ACT exp table off the critical path
            warm = miscp.tile([1, 64], f32, name="warm", tag="warm", bufs=1)
            nc.gpsimd.memset(warm[:], 0.0)
            nc.scalar.activation(warm[:], warm[:], Exp)

            proj_dst = {"q": qt, "k": kt, "v": vt}

            def proj_group(tq, pname, nch):
                """One [128,512] projection output; yields after each matmul."""
                dst, w = proj_dst[pname], w_sb["w" + pname]
                xq = xq_tiles[tq]
                ps = aps.tile([128, 512], f32, name=f"ps_{pname}{tq}{nch}", tag="yp",
                              bufs=2)
                for kc in range(KC):
                    nc.tensor.matmul(ps[:], w[:, kc, :],
                                     xq[:, kc, nch * 512:(nch + 1) * 512],
                                     start=(kc == 0), stop=(kc == KC - 1))
                    yield
                c0 = tq * 1024 + nch * 512
                nc.vector.tensor_copy(out=dst[:, c0:c0 + 512], in_=ps[:])
                yield

            def vtrans_unit(b, t):
                tp = aps.tile([128, 128], bf16, name="tp", tag="yp", bufs=2)
                nc.tensor.transpose(
                    tp[:], vt[:, b * S + t * 128: b * S + (t + 1) * 128], ident[:])
                nc.vector.tensor_copy(out=vp[:, b, t, 0:64], in_=tp[:, 0:64])
                nc.vector.tensor_copy(out=vp[:, b, t, 65:129], in_=tp[:, 64:128])
                yield

            def outproj_unit(b, qg, tt):
                """Y[tok-tile, :] for one 128-token tile (lhsT=O.T reused across od)."""
                q0 = b * S + qg * QG
                t0 = q0 + tt * 128
                for odc in range(2):
                    yp = aps.tile([128, 512], f32, name="yp", tag="yp", bufs=2)
                    nc.tensor.matmul(yp[:], ot[:, t0:t0 + 128],
                                     wo_sb[:, odc * 512:(odc + 1) * 512],
                                     start=True, stop=True)
                    ysb = ysbp.tile([128, 512], f32, name="ysb", tag="ysb")
                    nc.vector.tensor_copy(out=ysb[:], in_=yp[:])
                    nc.sync.dma_start(
                        y_d[t0:t0 + 128, odc * 512:(odc + 1) * 512], ysb[:])
                yield

            outproj_pending = []

            def attention_phase(b, qg, fill, pre=None):
                """One (batch, 512-query-group) phase; pulls from `fill` each kc.

                Software-pipelined: scores(kc+1) is emitted before attnV(kc) so
                the PE stays one step ahead of ACT and exp never waits. `pre(kc)`
                (if given) emits units that must precede attnV(kc) on the PE.
                """
                q0 = b * S + qg * QG

                def scores(kc):
                    k0 = b * S + kc * 128
                    sp = aps.tile([128, 2 * QG], f32, name="sp", tag="sp", bufs=2)
                    for h in range(2):
                        nc.tensor.matmul(
                            sp[:, h * QG:(h + 1) * QG],
                            kt[h * 64:(h + 1) * 64, k0:k0 + 128],
                            qt[h * 64:(h + 1) * 64, q0:q0 + QG],
                            start=True, stop=True)
                    return sp

                op = aps.tile([65, 2 * QG], f32, name="op", tag="op", bufs=1)
                sp_cur = scores(0)
                for kc in range(NKT):
                    sp_next = scores(kc + 1) if kc + 1 < NKT else None
                    pt = ptp.tile([128, 2 * QG], bf16, name="pt", tag="pt", bufs=3)
                    nc.scalar.activation(pt[:], sp_cur[:], Exp, scale=SCALE)
                    if pre is not None:
                        pre(kc)
                    for h in range(2):
                        nc.tensor.matmul(
                            op[:, h * QG:(h + 1) * QG],
                            vp[:, b, kc, h * 65:h * 65 + 65],
                            pt[:, h * QG:(h + 1) * QG],
                            start=(kc == 0), stop=(kc == NKT - 1))
                    sp_cur = sp_next
                    fill(kc)
                # stage op to SBUF (frees the PSUM slot for the next phase),
                # then normalize: O.T[:, q] /= denom[q] (denom = row 64, both heads)
                osb = miscp.tile([65, 2 * QG], f32, name="osb", tag="osb", bufs=2)
                nc.vector.tensor_copy(out=osb[:], in_=op[:])
                rr = miscp.tile([1, 2 * QG], f32, name="rr", tag="rr", bufs=2)
                nc.vector.reciprocal(rr[:], osb[64:65, :])
                rb = miscp.tile([64, 2 * QG], f32, name="rb", tag="rb", bufs=2)
                nc.gpsimd.partition_broadcast(rb[:], rr[:])
                for h in range(2):
                    nc.vector.tensor_mul(
                        out=ot[h * 64:(h + 1) * 64, q0:q0 + QG],
                        in0=osb[0:64, h * QG:(h + 1) * QG],
                        in1=rb[:, h * QG:(h + 1) * QG])
                outproj_pending.extend(
                    outproj_unit(b, qg, tt) for tt in range(QG // 128))

            def make_fill(stream, steps_per_call):
                state = {"it": iter(stream), "gen": None}

                def step():
                    """Advance the stream by one emitted chunk; False when done."""
                    while True:
                        if state["gen"] is None:
                            state["gen"] = next(state["it"], None)
                            if state["gen"] is None:
                                return False
                        if next(state["gen"], StopIteration) is StopIteration:
                            state["gen"] = None
                            continue
                        return True

                def fill(_kc):
                    for _ in range(steps_per_call):
                        if not step():
                            return

                def drain():
                    while step():
                        pass
                return fill, drain

            # ---- final structure: fragmented cross-phase/cross-region fill ----
            load_xq(0)
            load_weight("wv", wv_d, nc.scalar)
            load_weight("wq", wq_d, nc.scalar)
            load_xq(1)
            nc.scalar.dma_start(wo_sb[:], wo_d[:])
            for p, nch in (("k", 0), ("k", 1), ("v", 0), ("v", 1), ("q", 0)):
                for _ in proj_group(0, p, nch):
                    pass
            for t in range(NKT // 2):
                for _ in vtrans_unit(0, t):
                    pass
            for p in ("k", "v"):
                for nch in range(2):
                    for _ in proj_group(1, p, nch):
                        pass
            for t in range(NKT // 2, NKT):
                for _ in vtrans_unit(0, t):
                    pass

            load_xq(2)
            load_xq(3)
            fill1, drain1 = make_fill(
                [proj_group(0, "q", 1), proj_group(1, "q", 0), proj_group(1, "q", 1)]
                + [proj_group(2, p, nch) for p in ("k", "v") for nch in range(2)]
                + [proj_group(2, "q", 0)]
                + [proj_group(3, p, nch) for p in ("k", "v") for nch in range(2)]
                + [vtrans_unit(1, t) for t in range(NKT)],
                steps_per_call=3)
            for qg in range(NQG):
                attention_phase(0, qg, fill1)
            drain1()

            fill2a, _drain2a = make_fill(
                [proj_group(2, "q", 1), proj_group(3, "q", 0),
                 proj_group(3, "q", 1)], steps_per_call=3)
            fill2a_left = [9 * 3]

            def fill2(kc):
                if kc % 2 == 1:
                    if fill2a_left[0] > 0:
                        fill2a(kc)
                        fill2a_left[0] -= 3
                elif outproj_pending:
                    for _ in outproj_pending.pop(0):
                        pass
            for qg in range(NQG):
                attention_phase(1, qg, fill2)
            while outproj_pending:
                for _ in outproj_pending.pop(0):
                    pass
    nc.compile()
    return nc


def get_nc():
    if "nc" not in _CACHE:
        _CACHE["nc"] = _build_nc()
    return _CACHE["nc"]


def make_in_maps(hidden_states, Wq, Wk, Wv, Wo):
    bf = ml_dtypes.bfloat16
    X = np.ascontiguousarray(np.asarray(hidden_states, np.float32).reshape(TOK, D))
    xt = np.ascontiguousarray(X.T).astype(bf)
    Wq = np.asarray(Wq, np.float32)
    Wk = np.asarray(Wk, np.float32)
    Wv = np.asarray(Wv, np.float32)
    Wo = np.asarray(Wo, np.float32)
    in_maps = []
    for c in range(NCORES):
        sl = slice(c * HC, (c + 1) * HC)
        in_maps.append({
            "xt": xt,
            "wq": np.ascontiguousarray(Wq[:, sl]).astype(bf),
            "wk": np.ascontiguousarray(Wk[:, sl]).astype(bf),
            "wv": np.ascontiguousarray(Wv[:, sl]).astype(bf),
            "wo": np.ascontiguousarray(Wo[sl, :]).astype(bf),
        })
    return in_maps


def kernel(hidden_states, Wq, Wk, Wv, Wo, bo):
    from concourse.bass_utils import run_bass_kernel_spmd

    nc = get_nc()
    in_maps = make_in_maps(hidden_states, Wq, Wk, Wv, Wo)
    res = run_bass_kernel_spmd(nc, in_maps, list(range(NCORES)))
    _CACHE["last_result"] = res
    y = np.zeros((TOK, D), np.float32)
    for c in range(NCORES):
        y += res.results[c]["y"]
    out = y.reshape(B, S, D) + np.asarray(bo, np.float32)[None, None, :]
    return out.astype(np.float32)
